# revision 2
# baseline (speedup 1.0000x reference)
"""RNN-T JointNetwork kernel v3 for 8 Trainium2 NeuronCores.

logits = clip(tanh(enc@W_enc + b_enc [+] pred@W_pred + b_pred) @ W_out + b_out)

Sharding: data-parallel over T (each core takes T/8=32 frames, all B).

Device computes the bandwidth-heavy middle (99.5% of FLOPs):
  X^T[j, n]  = one-hot fp8-DoubleRow matmuls from hi/lo-split projections
  T16        = tanh(X) on ACT
  R8         = T16 - 0.7*X on DVE -> fp8  (tanh residual)
  PSUM[v, n] = R8 @ fp8(64*W_out): 3 perfectly-packed DR groups
               (d0/d1 = R rows 0..511; d2: s0 = R rows 512..639, s1 = pV hi/lo)
  eV[t]: K=32 DR matmul for ACT-copied pairs, or fused fp16 broadcast add in
  the DVE PSUM->SBUF copy for DVE-copied pairs.
Host prepares the tiny ends (projections = 0.5% of FLOPs, f32, following the
baseline's host input-prep pattern) and untransposes/scales the output.

Clip(+-15) is provably inactive (|logits| <= ~2).
"""
from contextlib import ExitStack

import ml_dtypes
import numpy as np

import concourse.bacc as bacc
import concourse.bass as bass  # noqa: F401
import concourse.tile as tile
from concourse import mybir
from concourse.bass_utils import run_bass_kernel_spmd

F32 = mybir.dt.float32
BF16 = mybir.dt.bfloat16
FP16 = mybir.dt.float16
FP8 = mybir.dt.float8e4
TANH = mybir.ActivationFunctionType.Tanh
COPY = mybir.ActivationFunctionType.Copy
DR = mybir.MatmulPerfMode.DoubleRow
MULT = mybir.AluOpType.mult
ADD = mybir.AluOpType.add

B, T, U = 4, 256, 64
DE, DP, DJ, V = 512, 640, 640, 1024
NCORES = 8
TL = T // NCORES           # 32 local t per core
BT = B * TL                # 128 (b,t) rows
NPB = TL * U               # 2048 pairs per batch
CC = 0.7
GS = 64.0
CAT = TL + U + 1           # 97 = pred-u rows, enc-t rows, bias row
KJ = 5

# copy-pair engine assignment: 'a' -> ACT (+eV matmul), 'd' -> DVE (fused eV).
# While X-work runs (stages 0-5) ACT takes more pairs (DVE busy with stt);
# in the X-free tail (stages 6-7) strict alternation keeps both engines fed.
import os
# per-pair half engines: 'ad' pairs overlap ACT/DVE inside the two psum bufs;
# a few 'aa' pairs skew work toward ACT (DVE carries the stt stream).
_PAT = os.environ.get("K3_PAT",
                      "ad ad ad ad ad ad ad aa ad ad ad ad ad ad ad aa")
if isinstance(_PAT, str):
    _PAT = _PAT.split()
PAIR_PAT = _PAT
_LOOK = int(os.environ.get("K3_LOOK", "1"))


def _build_nc():
    nc = bacc.Bacc("TRN2", target_bir_lowering=False, debug=False)
    smat_d = nc.dram_tensor("smat", [CAT, 2, NPB], FP8, kind="ExternalInput").ap()
    cats_d = nc.dram_tensor("cats", [CAT, B, 2, DJ], FP8, kind="ExternalInput").ap()
    estat_d = nc.dram_tensor("estat", [128, 2, V], FP8, kind="ExternalInput").ap()
    evT_d = nc.dram_tensor("evT", [128, 8, BT], FP16, kind="ExternalInput").ap()
    wst_d = nc.dram_tensor("wst", [128, 2, 2, V], FP8, kind="ExternalInput").ap()
    esel_d = nc.dram_tensor("esel", [128, 2, NPB], FP8, kind="ExternalInput").ap()
    usel_d = nc.dram_tensor("usel", [128, NPB], FP8, kind="ExternalInput").ap()
    d2st_d = nc.dram_tensor("d2st", [128, B, 2, V], FP8, kind="ExternalInput").ap()
    # output: 64*logits, fp16, [vt, v_local, b*2048 + pc2*1024 + t*64 + u]
    out_d = nc.dram_tensor("out", [8, 128, B * NPB], FP16, kind="ExternalOutput").ap()

    with tile.TileContext(nc) as tc, ExitStack() as ctx:
        const = ctx.enter_context(tc.tile_pool(name="const", bufs=1))

        # SP loads, need-order (X path first)
        smat = const.tile([CAT, 2, NPB], FP8, tag="smat")
        nc.sync.dma_start(smat[:], smat_d[:])
        cats8 = [const.tile([CAT, 2, DJ], FP8, tag=f"cats8_{b}", name=f"cats8_{b}")
                 for b in range(B)]
        for b in range(B):
            nc.sync.dma_start(cats8[b][:], cats_d[:, b, :, :])
        estat = const.tile([128, 2, V], FP8, tag="estat")
        nc.sync.dma_start(estat[:], estat_d[:])
        evT = const.tile([128, 8, BT], FP16, tag="evT")
        nc.sync.dma_start(evT[:], evT_d[:])

        # gpsimd SWDGE queue for the vocab-phase constants; tiny dependent
        # copy first so they can't race the critical SP loads on the DMA bus
        pgate = const.tile([1, 16], FP8, tag="pgate")
        nc.gpsimd.tensor_copy(pgate[:], cats8[B - 1][0:1, 0, 0:16])
        wst = const.tile([128, 2, 2, V], FP8, tag="wst")
        nc.gpsimd.dma_start(wst[:], wst_d[:])
        esel = const.tile([128, 2, NPB], FP8, tag="esel")
        nc.gpsimd.dma_start(esel[:], esel_d[:])
        d2st = [const.tile([128, 2, V], FP8, tag=f"d2st{b}", name=f"d2st{b}")
                for b in range(B)]
        for b in range(B):
            nc.gpsimd.dma_start(d2st[b][:], d2st_d[:, b, :, :])
        r8d2 = [const.tile([128, 2, NPB], FP8, tag=f"r8d2_{b}", name=f"r8d2_{b}")
                for b in range(B)]
        for b in range(B):
            nc.gpsimd.dma_start(r8d2[b][:, 1, :], usel_d[:])
        r8d = [[const.tile([128, 2, NPB], FP8, tag=f"r8d{d}_{b}", name=f"r8d{d}_{b}")
                for d in range(2)] for b in range(B)]

        tn_pool = ctx.enter_context(tc.tile_pool(name="tn", bufs=4))
        ost_pool = ctx.enter_context(tc.tile_pool(name="ost", bufs=8))
        xp_pool = ctx.enter_context(tc.tile_pool(name="xp", bufs=2, space="PSUM"))
        op_pool = ctx.enter_context(tc.tile_pool(name="op", bufs=2, space="PSUM"))

        unit = [0]

        def emit_x(b, jc, pc2):
            d, s = jc // 2, jc % 2
            n0 = pc2 * 1024
            xp = xp_pool.tile([128, 2, 512], F32, tag="xp")
            for q in range(2):
                nc.tensor.matmul(xp[:, q, :], cats8[b][:, :, jc * 128:(jc + 1) * 128],
                                 smat[:, :, n0 + q * 512:n0 + (q + 1) * 512],
                                 start=True, stop=True, perf_mode=DR)
            tn = tn_pool.tile([128, 1024], FP16, tag="tn")
            xv = xp[:].rearrange("p a b -> p (a b)")
            nc.scalar.activation(tn[:], xv, TANH)
            dst = (r8d2[b] if d == 2 else r8d[b][d])
            nc.vector.scalar_tensor_tensor(dst[:, s, n0:n0 + 1024], xv, -CC,
                                           tn[:], MULT, ADD)

        def emit_vocab_half(b, vt, pc2, ost, half, e):
            n0 = pc2 * 1024
            vs = slice(vt * 128, (vt + 1) * 128)
            op = op_pool.tile([128, 2, 512], F32, tag="op")
            for q in range(2):
                m0 = n0 + q * 512
                for d in range(2):
                    nc.tensor.matmul(op[:, q, :], wst[:, :, d, vs],
                                     r8d[b][d][:, :, m0:m0 + 512],
                                     start=(d == 0), stop=False, perf_mode=DR)
                nc.tensor.matmul(op[:, q, :], d2st[b][:, :, vs],
                                 r8d2[b][:, :, m0:m0 + 512],
                                 start=False, stop=(e == "d"), perf_mode=DR,
                                 skip_group_check=True)
                if e == "a":
                    nc.tensor.matmul(op[:, q, :],
                                     estat[b * TL:(b + 1) * TL, :, vs],
                                     esel[b * TL:(b + 1) * TL, :, m0:m0 + 512],
                                     start=False, stop=True, perf_mode=DR,
                                     skip_group_check=True,
                                     tile_position=(b * TL, 0))
            if e == "a":
                nc.scalar.activation(ost[:, half, :],
                                     op[:].rearrange("p a b -> p (a b)"), COPY)
            else:
                ev_in = evT[:, vt, b * TL + pc2 * 16:b * TL + pc2 * 16 + 16]
                nc.vector.scalar_tensor_tensor(
                    ost[:, half, :].rearrange("p (t u) -> p t u", t=16),
                    op[:].rearrange("p a (t u) -> p (a t) u", t=8),
                    1.0, ev_in.unsqueeze(2).broadcast_to((128, 16, 64)),
                    MULT, ADD)

        def emit_vocab_pair(b, vt0, pc2):
            e2 = PAIR_PAT[unit[0] % len(PAIR_PAT)]
            unit[0] += 1
            n0 = pc2 * 1024
            ost = ost_pool.tile([128, 2, 1024], FP16, tag="ost")
            emit_vocab_half(b, vt0, pc2, ost, 0, e2[0])
            emit_vocab_half(b, vt0 + 1, pc2, ost, 1, e2[1])
            c0 = b * NPB + n0
            nc.sync.dma_start(
                out_d[vt0:vt0 + 2, :, c0:c0 + 1024].rearrange("v p c -> p v c"),
                ost[:])

        def do_x(s):
            b, pc2 = s // 2, s % 2
            for jc in range(KJ):
                emit_x(b, jc, pc2)

        def do_v(s):
            b, pc2 = s // 2, s % 2
            for vt0 in (0, 2, 4, 6):
                emit_vocab_pair(b, vt0, pc2)

        import os as _os
        FINE = _os.environ.get("K3_FINE", "1") == "1"
        LOOK = _LOOK
        for s in range(LOOK):
            do_x(s)
        for s in range(8 - LOOK):
            if FINE:
                b, pc2 = s // 2, s % 2
                xb, xpc2 = (s + LOOK) // 2, (s + LOOK) % 2
                vts = [0, 2, 4, 6]
                for i in range(5):
                    if i < 4:
                        emit_vocab_pair(b, vts[i], pc2)
                    emit_x(xb, i, xpc2)
            else:
                do_v(s)
                do_x(s + LOOK)
        for s in range(8 - LOOK, 8):
            do_v(s)
    nc.compile()
    return nc


_NC = None


def _sel_consts():
    tt = (np.arange(NPB) // U)
    uu = (np.arange(NPB) % U)
    smat = np.zeros((CAT, NPB), np.float32)
    for u in range(U):
        smat[u, uu == u] = 1.0 / 32.0
    for t in range(TL):
        smat[U + t, tt == t] = 1.0 / 32.0
    smat[CAT - 1, :] = 1.0 / 32.0
    smat2 = np.stack([smat, smat], axis=1)              # [97, 2, NPB]
    esel = np.zeros((128, 2, NPB), np.float32)
    for p in range(128):
        esel[p, :, tt == (p % TL)] = 1.0
    usel = np.zeros((128, NPB), np.float32)
    for p in range(64):
        usel[p, uu == p] = 1.0
        usel[64 + p, uu == p] = 1.0
    return smat2, esel, usel


def kernel(encoder_out, predictor_out, W_enc, b_enc, W_pred, b_pred, W_out, b_out):
    global _NC
    if _NC is None:
        _NC = _build_nc()
    f8 = ml_dtypes.float8_e4m3fn
    f32 = np.float32

    def q8(x):
        return np.asarray(x, f32).astype(f8)

    Wo = np.asarray(W_out, f32)
    bsum = (np.asarray(b_enc, f32) + np.asarray(b_pred, f32))

    # stationary weights: fp8(64*W_out) rows j = d*256 + s*128 + p
    wst = (GS * Wo).reshape(KJ, 128, V)                 # [k, p, V], k = j//128
    wst4 = np.zeros((128, 2, 2, V), f32)                # d0/d1 groups
    for d in range(2):
        for s in range(2):
            wst4[:, s, d, :] = wst[2 * d + s]
    d2s0 = wst[4]                                       # rows 512..639

    smat2, esel, usel = _sel_consts()

    # host projections (f32) and hi/lo fp8 splits
    predj = np.asarray(predictor_out, f32).reshape(B * U, DP) @ np.asarray(W_pred, f32)
    predj32 = 32.0 * predj
    phi = q8(predj32)
    plo = q8(predj32 - phi.astype(f32))
    cb_hi = q8(32.0 * bsum)
    cb_lo = q8(32.0 * bsum - cb_hi.astype(f32))

    Q64 = (GS * CC) * (predj @ Wo)                      # [B*U, V]
    qhi = q8(Q64)
    qlo = q8(Q64 - qhi.astype(f32))
    d2st = np.zeros((128, B, 2, V), f32)
    d2st[:, :, 0, :] = d2s0[:, None, :]
    for b in range(B):
        d2st[0:64, b, 1, :] = qhi[b * U:(b + 1) * U].astype(f32)
        d2st[64:128, b, 1, :] = qlo[b * U:(b + 1) * U].astype(f32)

    shared = {
        "smat": smat2.astype(f8),
        "esel": esel.astype(f8),
        "usel": usel.astype(f8),
        "wst": wst4.astype(f8),
        "d2st": d2st.astype(f8),
    }

    enc_f = np.asarray(encoder_out, f32)
    We = np.asarray(W_enc, f32)
    in_maps = []
    for i in range(NCORES):
        encj = enc_f[:, i * TL:(i + 1) * TL, :].reshape(BT, DE) @ We   # [BT, DJ]
        encj32 = 32.0 * encj
        ehi = q8(encj32)
        elo = q8(encj32 - ehi.astype(f32))
        cats = np.zeros((CAT, B, 2, DJ), f32)
        for b in range(B):
            cats[0:U, b, 0, :] = phi[b * U:(b + 1) * U].astype(f32)
            cats[0:U, b, 1, :] = plo[b * U:(b + 1) * U].astype(f32)
            cats[U:U + TL, b, 0, :] = ehi[b * TL:(b + 1) * TL].astype(f32)
            cats[U:U + TL, b, 1, :] = elo[b * TL:(b + 1) * TL].astype(f32)
            cats[CAT - 1, b, 0, :] = cb_hi.astype(f32)
            cats[CAT - 1, b, 1, :] = cb_lo.astype(f32)

        P64 = GS * (CC * ((encj + bsum[None, :]) @ Wo) + np.asarray(b_out, f32))
        ehi_v = q8(P64)
        elo_v = q8(P64 - ehi_v.astype(f32))
        estat = np.stack([ehi_v.astype(f32), elo_v.astype(f32)], axis=1)
        evT = np.ascontiguousarray(
            P64.astype(np.float16).T.reshape(8, 128, BT).transpose(1, 0, 2))

        in_maps.append({**shared,
                        "cats": cats.astype(f8),
                        "estat": estat.astype(f8),
                        "evT": evT})
    res = run_bass_kernel_spmd(_NC, in_maps, core_ids=list(range(NCORES)))
    full = np.empty((B, T, U, V), np.float32)
    for i in range(NCORES):
        o = res.results[i]["out"].astype(np.float32) * f32(1.0 / GS)
        o = o.transpose(2, 0, 1).reshape(B, TL, U, 8 * 128)
        full[:, i * TL:(i + 1) * TL] = o
    return full


# revision 3
# speedup vs baseline: 1.0176x; 1.0176x over previous
"""RNN-T JointNetwork kernel v3 for 8 Trainium2 NeuronCores.

logits = clip(tanh(enc@W_enc + b_enc [+] pred@W_pred + b_pred) @ W_out + b_out)

Sharding: data-parallel over T (each core takes T/8=32 frames, all B).

Device computes the bandwidth-heavy middle (99.5% of FLOPs):
  X^T[j, n]  = one-hot fp8-DoubleRow matmuls from hi/lo-split projections
  T16        = tanh(X) on ACT
  R8         = T16 - 0.7*X on DVE -> fp8  (tanh residual)
  PSUM[v, n] = R8 @ fp8(64*W_out): 3 perfectly-packed DR groups
               (d0/d1 = R rows 0..511; d2: s0 = R rows 512..639, s1 = pV hi/lo)
  eV[t]: K=32 DR matmul for ACT-copied pairs, or fused fp16 broadcast add in
  the DVE PSUM->SBUF copy for DVE-copied pairs.
Host prepares the tiny ends (projections = 0.5% of FLOPs, f32, following the
baseline's host input-prep pattern) and untransposes/scales the output.

Clip(+-15) is provably inactive (|logits| <= ~2).
"""
from contextlib import ExitStack

import ml_dtypes
import numpy as np

import concourse.bacc as bacc
import concourse.bass as bass  # noqa: F401
import concourse.tile as tile
from concourse import mybir
from concourse.bass_utils import run_bass_kernel_spmd

F32 = mybir.dt.float32
BF16 = mybir.dt.bfloat16
FP16 = mybir.dt.float16
FP8 = mybir.dt.float8e4
TANH = mybir.ActivationFunctionType.Tanh
COPY = mybir.ActivationFunctionType.Copy
DR = mybir.MatmulPerfMode.DoubleRow
MULT = mybir.AluOpType.mult
ADD = mybir.AluOpType.add

B, T, U = 4, 256, 64
DE, DP, DJ, V = 512, 640, 640, 1024
NCORES = 8
TL = T // NCORES           # 32 local t per core
BT = B * TL                # 128 (b,t) rows
NPB = TL * U               # 2048 pairs per batch
CC = 0.7
GS = 64.0
CAT = TL + U + 1           # 97 = pred-u rows, enc-t rows, bias row
KJ = 5

# copy-pair engine assignment: 'a' -> ACT (+eV matmul), 'd' -> DVE (fused eV).
# While X-work runs (stages 0-5) ACT takes more pairs (DVE busy with stt);
# in the X-free tail (stages 6-7) strict alternation keeps both engines fed.
import os
# per-pair half engines: 'ad' pairs overlap ACT/DVE inside the two psum bufs;
# a few 'aa' pairs skew work toward ACT (DVE carries the stt stream).
_PAT = os.environ.get("K3_PAT",
                      "ad ad ad ad ad ad ad aa ad ad ad ad ad ad ad aa")
if isinstance(_PAT, str):
    _PAT = _PAT.split()
PAIR_PAT = _PAT
_LOOK = int(os.environ.get("K3_LOOK", "1"))


def _build_nc():
    nc = bacc.Bacc("TRN2", target_bir_lowering=False, debug=False)
    smat_d = nc.dram_tensor("smat", [CAT, 2, NPB], FP8, kind="ExternalInput").ap()
    cats_d = nc.dram_tensor("cats", [CAT, B, 2, DJ], FP8, kind="ExternalInput").ap()
    estat_d = nc.dram_tensor("estat", [128, 2, V], FP8, kind="ExternalInput").ap()
    evT_d = nc.dram_tensor("evT", [128, 8, BT], FP16, kind="ExternalInput").ap()
    wst_d = nc.dram_tensor("wst", [128, 2, 2, V], FP8, kind="ExternalInput").ap()
    esel_d = nc.dram_tensor("esel", [128, 2, NPB], FP8, kind="ExternalInput").ap()
    usel_d = nc.dram_tensor("usel", [128, NPB], FP8, kind="ExternalInput").ap()
    d2st_d = nc.dram_tensor("d2st", [128, B, 2, V], FP8, kind="ExternalInput").ap()
    # output: 64*logits, fp16, [vt, v_local, b*2048 + pc2*1024 + t*64 + u]
    out_d = nc.dram_tensor("out", [8, 128, B * NPB], FP16, kind="ExternalOutput").ap()

    with tile.TileContext(nc) as tc, ExitStack() as ctx:
        const = ctx.enter_context(tc.tile_pool(name="const", bufs=1))

        # SP loads, need-order (X path first)
        smat = const.tile([CAT, 2, NPB], FP8, tag="smat")
        cats8 = [const.tile([CAT, 2, DJ], FP8, tag=f"cats8_{b}", name=f"cats8_{b}")
                 for b in range(B)]
        nc.sync.dma_start(cats8[0][:], cats_d[:, 0, :, :])
        nc.sync.dma_start(smat[:, :, 0:1024], smat_d[:, :, 0:1024])
        nc.sync.dma_start(smat[:, :, 1024:NPB], smat_d[:, :, 1024:NPB])
        for b in range(1, B):
            nc.sync.dma_start(cats8[b][:], cats_d[:, b, :, :])
        estat = const.tile([128, 2, V], FP8, tag="estat")
        nc.sync.dma_start(estat[:], estat_d[:])
        evT = const.tile([128, 8, BT], FP16, tag="evT")
        nc.sync.dma_start(evT[:], evT_d[:])

        # gpsimd SWDGE queue for the vocab-phase constants; tiny dependent
        # copy first so they can't race the critical SP loads on the DMA bus
        pgate = const.tile([1, 16], FP8, tag="pgate")
        nc.gpsimd.tensor_copy(pgate[:], cats8[B - 1][0:1, 0, 0:16])
        wst = const.tile([128, 2, 2, V], FP8, tag="wst")
        nc.gpsimd.dma_start(wst[:], wst_d[:])
        esel = const.tile([128, 2, NPB], FP8, tag="esel")
        nc.gpsimd.dma_start(esel[:], esel_d[:])
        d2st = [const.tile([128, 2, V], FP8, tag=f"d2st{b}", name=f"d2st{b}")
                for b in range(B)]
        for b in range(B):
            nc.gpsimd.dma_start(d2st[b][:], d2st_d[:, b, :, :])
        r8d2 = [const.tile([128, 2, NPB], FP8, tag=f"r8d2_{b}", name=f"r8d2_{b}")
                for b in range(B)]
        for b in range(B):
            nc.gpsimd.dma_start(r8d2[b][:, 1, :], usel_d[:])
        r8d = [[const.tile([128, 2, NPB], FP8, tag=f"r8d{d}_{b}", name=f"r8d{d}_{b}")
                for d in range(2)] for b in range(B)]

        tn_pool = ctx.enter_context(tc.tile_pool(name="tn", bufs=6))
        ost_pool = ctx.enter_context(tc.tile_pool(name="ost", bufs=10))
        xp_pool = ctx.enter_context(tc.tile_pool(name="xp", bufs=2, space="PSUM"))
        op_pool = ctx.enter_context(tc.tile_pool(name="op", bufs=2, space="PSUM"))

        unit = [0]

        def emit_x(b, jc, pc2):
            d, s = jc // 2, jc % 2
            n0 = pc2 * 1024
            xp = xp_pool.tile([128, 2, 512], F32, tag="xp")
            for q in range(2):
                nc.tensor.matmul(xp[:, q, :], cats8[b][:, :, jc * 128:(jc + 1) * 128],
                                 smat[:, :, n0 + q * 512:n0 + (q + 1) * 512],
                                 start=True, stop=True, perf_mode=DR)
            tn = tn_pool.tile([128, 1024], FP16, tag="tn")
            xv = xp[:].rearrange("p a b -> p (a b)")
            nc.scalar.activation(tn[:], xv, TANH)
            dst = (r8d2[b] if d == 2 else r8d[b][d])
            nc.vector.scalar_tensor_tensor(dst[:, s, n0:n0 + 1024], xv, -CC,
                                           tn[:], MULT, ADD)

        def emit_vocab_half(b, vt, pc2, ost, half, e):
            n0 = pc2 * 1024
            vs = slice(vt * 128, (vt + 1) * 128)
            op = op_pool.tile([128, 2, 512], F32, tag="op")
            for q in range(2):
                m0 = n0 + q * 512
                for d in range(2):
                    nc.tensor.matmul(op[:, q, :], wst[:, :, d, vs],
                                     r8d[b][d][:, :, m0:m0 + 512],
                                     start=(d == 0), stop=False, perf_mode=DR)
                nc.tensor.matmul(op[:, q, :], d2st[b][:, :, vs],
                                 r8d2[b][:, :, m0:m0 + 512],
                                 start=False, stop=(e == "d"), perf_mode=DR,
                                 skip_group_check=True)
                if e == "a":
                    nc.tensor.matmul(op[:, q, :],
                                     estat[b * TL:(b + 1) * TL, :, vs],
                                     esel[b * TL:(b + 1) * TL, :, m0:m0 + 512],
                                     start=False, stop=True, perf_mode=DR,
                                     skip_group_check=True,
                                     tile_position=(b * TL, 0))
            if e == "a":
                nc.scalar.activation(ost[:, half, :],
                                     op[:].rearrange("p a b -> p (a b)"), COPY)
            else:
                ev_in = evT[:, vt, b * TL + pc2 * 16:b * TL + pc2 * 16 + 16]
                nc.vector.scalar_tensor_tensor(
                    ost[:, half, :].rearrange("p (t u) -> p t u", t=16),
                    op[:].rearrange("p a (t u) -> p (a t) u", t=8),
                    1.0, ev_in.unsqueeze(2).broadcast_to((128, 16, 64)),
                    MULT, ADD)

        def emit_vocab_pair(b, vt0, pc2):
            e2 = PAIR_PAT[unit[0] % len(PAIR_PAT)]
            unit[0] += 1
            n0 = pc2 * 1024
            ost = ost_pool.tile([128, 2, 1024], FP16, tag="ost")
            emit_vocab_half(b, vt0, pc2, ost, 0, e2[0])
            emit_vocab_half(b, vt0 + 1, pc2, ost, 1, e2[1])
            c0 = b * NPB + n0
            nc.sync.dma_start(
                out_d[vt0:vt0 + 2, :, c0:c0 + 1024].rearrange("v p c -> p v c"),
                ost[:])

        def do_x(s):
            b, pc2 = s // 2, s % 2
            for jc in range(KJ):
                emit_x(b, jc, pc2)

        def do_v(s):
            b, pc2 = s // 2, s % 2
            for vt0 in (0, 2, 4, 6):
                emit_vocab_pair(b, vt0, pc2)

        import os as _os
        FINE = _os.environ.get("K3_FINE", "1") == "1"
        LOOK = _LOOK
        for s in range(LOOK):
            do_x(s)
        for s in range(8 - LOOK):
            if FINE:
                b, pc2 = s // 2, s % 2
                xb, xpc2 = (s + LOOK) // 2, (s + LOOK) % 2
                vts = [0, 2, 4, 6]
                for i in range(5):
                    if i < 4:
                        emit_vocab_pair(b, vts[i], pc2)
                    emit_x(xb, i, xpc2)
            else:
                do_v(s)
                do_x(s + LOOK)
        for s in range(8 - LOOK, 8):
            do_v(s)
    nc.compile()
    return nc


_NC = None


def _sel_consts():
    tt = (np.arange(NPB) // U)
    uu = (np.arange(NPB) % U)
    smat = np.zeros((CAT, NPB), np.float32)
    for u in range(U):
        smat[u, uu == u] = 1.0 / 32.0
    for t in range(TL):
        smat[U + t, tt == t] = 1.0 / 32.0
    smat[CAT - 1, :] = 1.0 / 32.0
    smat2 = np.stack([smat, smat], axis=1)              # [97, 2, NPB]
    esel = np.zeros((128, 2, NPB), np.float32)
    for p in range(128):
        esel[p, :, tt == (p % TL)] = 1.0
    usel = np.zeros((128, NPB), np.float32)
    for p in range(64):
        usel[p, uu == p] = 1.0
        usel[64 + p, uu == p] = 1.0
    return smat2, esel, usel


def kernel(encoder_out, predictor_out, W_enc, b_enc, W_pred, b_pred, W_out, b_out):
    global _NC
    if _NC is None:
        _NC = _build_nc()
    f8 = ml_dtypes.float8_e4m3fn
    f32 = np.float32

    def q8(x):
        return np.asarray(x, f32).astype(f8)

    Wo = np.asarray(W_out, f32)
    bsum = (np.asarray(b_enc, f32) + np.asarray(b_pred, f32))

    # stationary weights: fp8(64*W_out) rows j = d*256 + s*128 + p
    wst = (GS * Wo).reshape(KJ, 128, V)                 # [k, p, V], k = j//128
    wst4 = np.zeros((128, 2, 2, V), f32)                # d0/d1 groups
    for d in range(2):
        for s in range(2):
            wst4[:, s, d, :] = wst[2 * d + s]
    d2s0 = wst[4]                                       # rows 512..639

    smat2, esel, usel = _sel_consts()

    # host projections (f32) and hi/lo fp8 splits
    predj = np.asarray(predictor_out, f32).reshape(B * U, DP) @ np.asarray(W_pred, f32)
    predj32 = 32.0 * predj
    phi = q8(predj32)
    plo = q8(predj32 - phi.astype(f32))
    cb_hi = q8(32.0 * bsum)
    cb_lo = q8(32.0 * bsum - cb_hi.astype(f32))

    Q64 = (GS * CC) * (predj @ Wo)                      # [B*U, V]
    qhi = q8(Q64)
    qlo = q8(Q64 - qhi.astype(f32))
    d2st = np.zeros((128, B, 2, V), f32)
    d2st[:, :, 0, :] = d2s0[:, None, :]
    for b in range(B):
        d2st[0:64, b, 1, :] = qhi[b * U:(b + 1) * U].astype(f32)
        d2st[64:128, b, 1, :] = qlo[b * U:(b + 1) * U].astype(f32)

    shared = {
        "smat": smat2.astype(f8),
        "esel": esel.astype(f8),
        "usel": usel.astype(f8),
        "wst": wst4.astype(f8),
        "d2st": d2st.astype(f8),
    }

    enc_f = np.asarray(encoder_out, f32)
    We = np.asarray(W_enc, f32)
    in_maps = []
    for i in range(NCORES):
        encj = enc_f[:, i * TL:(i + 1) * TL, :].reshape(BT, DE) @ We   # [BT, DJ]
        encj32 = 32.0 * encj
        ehi = q8(encj32)
        elo = q8(encj32 - ehi.astype(f32))
        cats = np.zeros((CAT, B, 2, DJ), f32)
        for b in range(B):
            cats[0:U, b, 0, :] = phi[b * U:(b + 1) * U].astype(f32)
            cats[0:U, b, 1, :] = plo[b * U:(b + 1) * U].astype(f32)
            cats[U:U + TL, b, 0, :] = ehi[b * TL:(b + 1) * TL].astype(f32)
            cats[U:U + TL, b, 1, :] = elo[b * TL:(b + 1) * TL].astype(f32)
            cats[CAT - 1, b, 0, :] = cb_hi.astype(f32)
            cats[CAT - 1, b, 1, :] = cb_lo.astype(f32)

        P64 = GS * (CC * ((encj + bsum[None, :]) @ Wo) + np.asarray(b_out, f32))
        ehi_v = q8(P64)
        elo_v = q8(P64 - ehi_v.astype(f32))
        estat = np.stack([ehi_v.astype(f32), elo_v.astype(f32)], axis=1)
        evT = np.ascontiguousarray(
            P64.astype(np.float16).T.reshape(8, 128, BT).transpose(1, 0, 2))

        in_maps.append({**shared,
                        "cats": cats.astype(f8),
                        "estat": estat.astype(f8),
                        "evT": evT})
    res = run_bass_kernel_spmd(_NC, in_maps, core_ids=list(range(NCORES)))
    full = np.empty((B, T, U, V), np.float32)
    for i in range(NCORES):
        o = res.results[i]["out"].astype(np.float32) * f32(1.0 / GS)
        o = o.transpose(2, 0, 1).reshape(B, TL, U, 8 * 128)
        full[:, i * TL:(i + 1) * TL] = o
    return full


# revision 6
# speedup vs baseline: 1.0348x; 1.0169x over previous
"""RNN-T JointNetwork kernel v3 for 8 Trainium2 NeuronCores.

logits = clip(tanh(enc@W_enc + b_enc [+] pred@W_pred + b_pred) @ W_out + b_out)

Sharding: data-parallel over T (each core takes T/8=32 frames, all B).

Device computes the bandwidth-heavy middle (99.5% of FLOPs):
  X^T[j, n]  = one-hot fp8-DoubleRow matmuls from hi/lo-split projections
  T16        = tanh(X) on ACT
  R8         = T16 - 0.7*X on DVE -> fp8  (tanh residual)
  PSUM[v, n] = R8 @ fp8(64*W_out): 3 perfectly-packed DR groups
               (d0/d1 = R rows 0..511; d2: s0 = R rows 512..639, s1 = pV hi/lo)
  eV[t]: K=32 DR matmul for ACT-copied pairs, or fused fp16 broadcast add in
  the DVE PSUM->SBUF copy for DVE-copied pairs.
Host prepares the tiny ends (projections = 0.5% of FLOPs, f32, following the
baseline's host input-prep pattern) and untransposes/scales the output.

Clip(+-15) is provably inactive (|logits| <= ~2).
"""
from contextlib import ExitStack

import ml_dtypes
import numpy as np

import concourse.bacc as bacc
import concourse.bass as bass  # noqa: F401
import concourse.tile as tile
from concourse import mybir
from concourse.bass_utils import run_bass_kernel_spmd

F32 = mybir.dt.float32
BF16 = mybir.dt.bfloat16
FP16 = mybir.dt.float16
FP8 = mybir.dt.float8e4
TANH = mybir.ActivationFunctionType.Tanh
COPY = mybir.ActivationFunctionType.Copy
DR = mybir.MatmulPerfMode.DoubleRow
MULT = mybir.AluOpType.mult
ADD = mybir.AluOpType.add

B, T, U = 4, 256, 64
DE, DP, DJ, V = 512, 640, 640, 1024
NCORES = 8
TL = T // NCORES           # 32 local t per core
BT = B * TL                # 128 (b,t) rows
NPB = TL * U               # 2048 pairs per batch
CC = 0.7
GS = 64.0
CAT = TL + U + 1           # 97 = pred-u rows, enc-t rows, bias row
KJ = 5

# copy-pair engine assignment: 'a' -> ACT (+eV matmul), 'd' -> DVE (fused eV).
# While X-work runs (stages 0-5) ACT takes more pairs (DVE busy with stt);
# in the X-free tail (stages 6-7) strict alternation keeps both engines fed.
import os
# per-pair half engines: 'ad' pairs overlap ACT/DVE inside the two psum bufs;
# a few 'aa' pairs skew work toward ACT (DVE carries the stt stream).
_PAT = os.environ.get("K3_PAT",
                      "ad ad ad aa ad ad ad ad ad ad ad aa ad ad ad ad")
if isinstance(_PAT, str):
    _PAT = _PAT.split()
PAIR_PAT = _PAT
_LOOK = int(os.environ.get("K3_LOOK", "1"))


def _build_nc():
    nc = bacc.Bacc("TRN2", target_bir_lowering=False, debug=False)
    boot_d = nc.dram_tensor("boot", [CAT, 2, 1024 + DJ], FP8, kind="ExternalInput").ap()
    smat_d = nc.dram_tensor("smat", [CAT, 2, 1024], FP8, kind="ExternalInput").ap()
    cats_d = nc.dram_tensor("cats", [CAT, B, 2, DJ], FP8, kind="ExternalInput").ap()
    estat_d = nc.dram_tensor("estat", [128, 2, V], FP8, kind="ExternalInput").ap()
    evT_d = nc.dram_tensor("evT", [128, 8, BT], FP16, kind="ExternalInput").ap()
    wst_d = nc.dram_tensor("wst", [128, 2, 2, V], FP8, kind="ExternalInput").ap()
    esel_d = nc.dram_tensor("esel", [128, 2, NPB], FP8, kind="ExternalInput").ap()
    usel_d = nc.dram_tensor("usel", [128, NPB], FP8, kind="ExternalInput").ap()
    d2st_d = nc.dram_tensor("d2st", [128, B, 2, V], FP8, kind="ExternalInput").ap()
    # output: 64*logits, fp16, [vt, v_local, b*2048 + pc2*1024 + t*64 + u]
    out_d = nc.dram_tensor("out", [8, 128, B * NPB], FP16, kind="ExternalOutput").ap()

    with tile.TileContext(nc) as tc, ExitStack() as ctx:
        const = ctx.enter_context(tc.tile_pool(name="const", bufs=1))

        # SP loads, need-order: one boot DMA covers the first X unit's needs
        boot = const.tile([CAT, 2, 1024 + DJ], FP8, tag="boot")
        nc.sync.dma_start(boot[:], boot_d[:])
        smatB = const.tile([CAT, 2, 1024], FP8, tag="smatB")
        nc.sync.dma_start(smatB[:], smat_d[:])
        cats8 = [None] + [const.tile([CAT, 2, DJ], FP8, tag=f"cats8_{b}",
                                     name=f"cats8_{b}") for b in range(1, B)]
        for b in range(1, B):
            nc.sync.dma_start(cats8[b][:], cats_d[:, b, :, :])

        def cats_ap(b):
            return boot[:, :, 1024:1024 + DJ] if b == 0 else cats8[b][:]

        def smat_ap(pc2):
            return boot[:, :, 0:1024] if pc2 == 0 else smatB[:]
        estat = const.tile([128, 2, V], FP8, tag="estat")
        nc.sync.dma_start(estat[:], estat_d[:])
        evT = const.tile([128, 8, BT], FP16, tag="evT")
        nc.sync.dma_start(evT[:], evT_d[:])

        # gpsimd SWDGE queue for the vocab-phase constants; tiny dependent
        # copy first so they can't race the critical SP loads on the DMA bus
        pgate = const.tile([1, 16], FP8, tag="pgate")
        nc.gpsimd.tensor_copy(pgate[:], cats8[B - 1][0:1, 0, 0:16])
        wst = const.tile([128, 2, 2, V], FP8, tag="wst")
        nc.gpsimd.dma_start(wst[:], wst_d[:])
        esel = const.tile([128, 2, NPB], FP8, tag="esel")
        nc.gpsimd.dma_start(esel[:], esel_d[:])
        d2st = [const.tile([128, 2, V], FP8, tag=f"d2st{b}", name=f"d2st{b}")
                for b in range(B)]
        for b in range(B):
            nc.gpsimd.dma_start(d2st[b][:], d2st_d[:, b, :, :])
        r8d2 = [const.tile([128, 2, NPB], FP8, tag=f"r8d2_{b}", name=f"r8d2_{b}")
                for b in range(B)]
        for b in range(B):
            nc.gpsimd.dma_start(r8d2[b][:, 1, :], usel_d[:])
        r8d = [[const.tile([128, 2, NPB], FP8, tag=f"r8d{d}_{b}", name=f"r8d{d}_{b}")
                for d in range(2)] for b in range(B)]

        tn_pool = ctx.enter_context(tc.tile_pool(name="tn", bufs=6))
        ost_pool = ctx.enter_context(tc.tile_pool(name="ost", bufs=10))
        xp_pool = ctx.enter_context(tc.tile_pool(name="xp", bufs=2, space="PSUM"))
        op_pool = ctx.enter_context(tc.tile_pool(name="op", bufs=2, space="PSUM"))

        unit = [0]

        def emit_x(b, jc, pc2):
            d, s = jc // 2, jc % 2
            n0 = pc2 * 1024
            xp = xp_pool.tile([128, 2, 512], F32, tag="xp")
            ca, sa = cats_ap(b), smat_ap(pc2)
            for q in range(2):
                nc.tensor.matmul(xp[:, q, :], ca[:, :, jc * 128:(jc + 1) * 128],
                                 sa[:, :, q * 512:(q + 1) * 512],
                                 start=True, stop=True, perf_mode=DR)
            tn = tn_pool.tile([128, 1024], FP16, tag="tn")
            xv = xp[:].rearrange("p a b -> p (a b)")
            nc.scalar.activation(tn[:], xv, TANH)
            dst = (r8d2[b] if d == 2 else r8d[b][d])
            nc.vector.scalar_tensor_tensor(dst[:, s, n0:n0 + 1024], xv, -CC,
                                           tn[:], MULT, ADD)

        def emit_vocab_half(b, vt, pc2, ost, half, e):
            n0 = pc2 * 1024
            vs = slice(vt * 128, (vt + 1) * 128)
            op = op_pool.tile([128, 2, 512], F32, tag="op")
            for q in range(2):
                m0 = n0 + q * 512
                for d in range(2):
                    nc.tensor.matmul(op[:, q, :], wst[:, :, d, vs],
                                     r8d[b][d][:, :, m0:m0 + 512],
                                     start=(d == 0), stop=False, perf_mode=DR)
                nc.tensor.matmul(op[:, q, :], d2st[b][:, :, vs],
                                 r8d2[b][:, :, m0:m0 + 512],
                                 start=False, stop=(e == "d"), perf_mode=DR,
                                 skip_group_check=True)
                if e == "a":
                    nc.tensor.matmul(op[:, q, :],
                                     estat[b * TL:(b + 1) * TL, :, vs],
                                     esel[b * TL:(b + 1) * TL, :, m0:m0 + 512],
                                     start=False, stop=True, perf_mode=DR,
                                     skip_group_check=True,
                                     tile_position=(b * TL, 0))
            if e == "a":
                nc.scalar.activation(ost[:, half, :],
                                     op[:].rearrange("p a b -> p (a b)"), COPY)
            else:
                ev_in = evT[:, vt, b * TL + pc2 * 16:b * TL + pc2 * 16 + 16]
                nc.vector.scalar_tensor_tensor(
                    ost[:, half, :].rearrange("p (t u) -> p t u", t=16),
                    op[:].rearrange("p a (t u) -> p (a t) u", t=8),
                    1.0, ev_in.unsqueeze(2).broadcast_to((128, 16, 64)),
                    MULT, ADD)

        def emit_vocab_pair(b, vt0, pc2, split_dma=False):
            e2 = PAIR_PAT[unit[0] % len(PAIR_PAT)]
            unit[0] += 1
            n0 = pc2 * 1024
            ost = ost_pool.tile([128, 2, 1024], FP16, tag="ost")
            c0 = b * NPB + n0
            if split_dma:
                # tail stages: ship each half as soon as its copy lands
                emit_vocab_half(b, vt0, pc2, ost, 0, e2[0])
                nc.sync.dma_start(out_d[vt0, :, c0:c0 + 1024], ost[:, 0, :])
                emit_vocab_half(b, vt0 + 1, pc2, ost, 1, e2[1])
                nc.sync.dma_start(out_d[vt0 + 1, :, c0:c0 + 1024], ost[:, 1, :])
            else:
                emit_vocab_half(b, vt0, pc2, ost, 0, e2[0])
                emit_vocab_half(b, vt0 + 1, pc2, ost, 1, e2[1])
                nc.sync.dma_start(
                    out_d[vt0:vt0 + 2, :, c0:c0 + 1024].rearrange("v p c -> p v c"),
                    ost[:])

        def do_x(s):
            b, pc2 = s // 2, s % 2
            for jc in range(KJ):
                emit_x(b, jc, pc2)

        def do_v(s):
            b, pc2 = s // 2, s % 2
            for vt0 in (0, 2, 4, 6):
                emit_vocab_pair(b, vt0, pc2, split_dma=(s >= 6))

        import os as _os
        FINE = _os.environ.get("K3_FINE", "1") == "1"
        LOOK = _LOOK
        for s in range(LOOK):
            do_x(s)
        for s in range(8 - LOOK):
            if FINE:
                b, pc2 = s // 2, s % 2
                xb, xpc2 = (s + LOOK) // 2, (s + LOOK) % 2
                vts = [0, 2, 4, 6]
                for i in range(5):
                    if i < 4:
                        emit_vocab_pair(b, vts[i], pc2)
                    emit_x(xb, i, xpc2)
            else:
                do_v(s)
                do_x(s + LOOK)
        for s in range(8 - LOOK, 8):
            do_v(s)
    nc.compile()
    return nc


_NC = None


def _sel_consts():
    tt = (np.arange(NPB) // U)
    uu = (np.arange(NPB) % U)
    smat = np.zeros((CAT, NPB), np.float32)
    for u in range(U):
        smat[u, uu == u] = 1.0 / 32.0
    for t in range(TL):
        smat[U + t, tt == t] = 1.0 / 32.0
    smat[CAT - 1, :] = 1.0 / 32.0
    smat2 = np.stack([smat, smat], axis=1)              # [97, 2, NPB]
    esel = np.zeros((128, 2, NPB), np.float32)
    for p in range(128):
        esel[p, :, tt == (p % TL)] = 1.0
    usel = np.zeros((128, NPB), np.float32)
    for p in range(64):
        usel[p, uu == p] = 1.0
        usel[64 + p, uu == p] = 1.0
    return smat2, esel, usel


def kernel(encoder_out, predictor_out, W_enc, b_enc, W_pred, b_pred, W_out, b_out):
    global _NC
    if _NC is None:
        _NC = _build_nc()
    f8 = ml_dtypes.float8_e4m3fn
    f32 = np.float32

    def q8(x):
        return np.asarray(x, f32).astype(f8)

    Wo = np.asarray(W_out, f32)
    bsum = (np.asarray(b_enc, f32) + np.asarray(b_pred, f32))

    # stationary weights: fp8(64*W_out) rows j = d*256 + s*128 + p
    wst = (GS * Wo).reshape(KJ, 128, V)                 # [k, p, V], k = j//128
    wst4 = np.zeros((128, 2, 2, V), f32)                # d0/d1 groups
    for d in range(2):
        for s in range(2):
            wst4[:, s, d, :] = wst[2 * d + s]
    d2s0 = wst[4]                                       # rows 512..639

    smat2, esel, usel = _sel_consts()

    # host projections (f32) and hi/lo fp8 splits
    predj = np.asarray(predictor_out, f32).reshape(B * U, DP) @ np.asarray(W_pred, f32)
    predj32 = 32.0 * predj
    phi = q8(predj32)
    plo = q8(predj32 - phi.astype(f32))
    cb_hi = q8(32.0 * bsum)
    cb_lo = q8(32.0 * bsum - cb_hi.astype(f32))

    Q64 = (GS * CC) * (predj @ Wo)                      # [B*U, V]
    qhi = q8(Q64)
    qlo = q8(Q64 - qhi.astype(f32))
    d2st = np.zeros((128, B, 2, V), f32)
    d2st[:, :, 0, :] = d2s0[:, None, :]
    for b in range(B):
        d2st[0:64, b, 1, :] = qhi[b * U:(b + 1) * U].astype(f32)
        d2st[64:128, b, 1, :] = qlo[b * U:(b + 1) * U].astype(f32)

    shared = {
        "smat": smat2[:, :, 1024:].astype(f8),
        "esel": esel.astype(f8),
        "usel": usel.astype(f8),
        "wst": wst4.astype(f8),
        "d2st": d2st.astype(f8),
    }

    enc_f = np.asarray(encoder_out, f32)
    We = np.asarray(W_enc, f32)
    smat_lo = smat2[:, :, 0:1024].astype(f8)
    in_maps = []
    for i in range(NCORES):
        encj = enc_f[:, i * TL:(i + 1) * TL, :].reshape(BT, DE) @ We   # [BT, DJ]
        encj32 = 32.0 * encj
        ehi = q8(encj32)
        elo = q8(encj32 - ehi.astype(f32))
        cats = np.zeros((CAT, B, 2, DJ), f32)
        for b in range(B):
            cats[0:U, b, 0, :] = phi[b * U:(b + 1) * U].astype(f32)
            cats[0:U, b, 1, :] = plo[b * U:(b + 1) * U].astype(f32)
            cats[U:U + TL, b, 0, :] = ehi[b * TL:(b + 1) * TL].astype(f32)
            cats[U:U + TL, b, 1, :] = elo[b * TL:(b + 1) * TL].astype(f32)
            cats[CAT - 1, b, 0, :] = cb_hi.astype(f32)
            cats[CAT - 1, b, 1, :] = cb_lo.astype(f32)

        P64 = GS * (CC * ((encj + bsum[None, :]) @ Wo) + np.asarray(b_out, f32))
        ehi_v = q8(P64)
        elo_v = q8(P64 - ehi_v.astype(f32))
        estat = np.stack([ehi_v.astype(f32), elo_v.astype(f32)], axis=1)
        evT = np.ascontiguousarray(
            P64.astype(np.float16).T.reshape(8, 128, BT).transpose(1, 0, 2))

        boot = np.concatenate([smat_lo.astype(f32),
                               cats[:, 0, :, :]], axis=2)
        in_maps.append({**shared,
                        "boot": boot.astype(f8),
                        "cats": cats.astype(f8),
                        "estat": estat.astype(f8),
                        "evT": evT})
    res = run_bass_kernel_spmd(_NC, in_maps, core_ids=list(range(NCORES)))
    full = np.empty((B, T, U, V), np.float32)
    for i in range(NCORES):
        o = res.results[i]["out"].astype(np.float32) * f32(1.0 / GS)
        o = o.transpose(2, 0, 1).reshape(B, TL, U, 8 * 128)
        full[:, i * TL:(i + 1) * TL] = o
    return full
